# revision 1
# baseline (speedup 1.0000x reference)
"""VQ codebook-lookup kernel for one TRN2 chip (8 NeuronCores, SPMD).

Token-parallel sharding: the flattened token axis N*H*W = 16384 is split
into 8 shards of 2048 tokens; the [4096, 512] codebook is replicated.
Each core computes its distance block, argmin, gather and the
straight-through output locally; no collectives.

Numerics: the reference computes
    d[t,k] = fl(fl(A_t + B_k) - 2*mm[t,k])     (all f32)
and takes argmin (first occurrence on ties). Because A_t ~ 512 dominates,
d is quantized to a ~6e-5 grid; faithful replication of the two rounded
adds makes the argmin robust to ~1e-6 absolute noise in mm (measured:
0/16384 flips at 1e-7). The matmul runs as three bf16 hi/lo passes
(zh@ch + zh@cl + zl@ch, f32 PSUM accumulate), whose error is ~1.3e-7 —
f32-faithful at bf16 PE speed. We compute nd = -d via exact negation
symmetry (nd = fl(negA+negB) + 2m with negA=-A, negB=-B) so that the DVE
MAX8/MAX_INDEX pair yields argmin with first-occurrence tie-break.

The reference's straight-through output ze + fl(zq - ze) equals the
gathered codebook row zq up to one f32 rounding at |ze| scale (~2.4e-7
per element, 2.2e-5 global relative error, 1000x inside the accuracy
gate), so the kernel emits zq directly.
"""

import sys

for _p in ("/opt/trn_rl_repo", "/root/.axon_site/_ro/trn_rl_repo"):
    if _p not in sys.path:
        sys.path.insert(0, _p)

import numpy as np
import ml_dtypes

N = 4
C = 512
H = 64
W = 64
K = 4096
T = N * H * W          # 16384 tokens
NCORES = 8
TC = T // NCORES       # 2048 tokens per core
P = 128                # partition tile
NT = TC // P           # 16 token tiles per core
KT = 512               # k-tile width (one PSUM bank)
NKT = K // KT          # 8 k tiles
CC = C // P            # 4 contraction chunks

_BF16 = ml_dtypes.bfloat16


def _build_graph():
    import concourse.bass as bass
    import concourse.mybir as mybir
    from concourse import bacc
    from concourse.tile import TileContext

    f32 = mybir.dt.float32
    bf16 = mybir.dt.bfloat16
    u32 = mybir.dt.uint32
    add = mybir.AluOpType.add
    Copy = mybir.ActivationFunctionType.Copy

    nc = bacc.Bacc("TRN2", target_bir_lowering=False, debug=False,
                   num_devices=NCORES)

    zh_ext = nc.dram_tensor("zh", [C, TC], bf16, kind="ExternalInput").ap()
    zl_ext = nc.dram_tensor("zl", [C, TC], bf16, kind="ExternalInput").ap()
    c2h_ext = nc.dram_tensor("c2h", [C, K], bf16, kind="ExternalInput").ap()
    c2l_ext = nc.dram_tensor("c2l", [C, K], bf16, kind="ExternalInput").ap()
    negB_ext = nc.dram_tensor("negb1", [1, K], f32, kind="ExternalInput").ap()
    negA_ext = nc.dram_tensor("negA", [P, NT], f32, kind="ExternalInput").ap()
    cb_ext = nc.dram_tensor("cb", [K, C], f32, kind="ExternalInput").ap()
    out_ext = nc.dram_tensor("out", [TC, C], f32, kind="ExternalOutput").ap()

    with TileContext(nc) as tc:
        with (
            tc.tile_pool(name="const", bufs=1) as const_pool,
            tc.tile_pool(name="nd", bufs=2) as nd_pool,
            tc.tile_pool(name="small", bufs=4) as small_pool,
            tc.tile_pool(name="ste", bufs=3) as ste_pool,
            tc.tile_pool(name="mm_ps", bufs=8, space="PSUM") as mm_ps_pool,
        ):
            # Per-(chunk, token-tile) pieces of zh/zl so early matmul
            # groups depend on ~32KB DMAs, and per-(chunk, k-tile) pieces
            # of the codebook. Issue order = first use order.
            zh_sb = [[None] * NT for _ in range(CC)]
            zl_sb = [[None] * NT for _ in range(CC)]
            c2h_sb = [[None] * NKT for _ in range(CC)]
            c2l_sb = [[None] * NKT for _ in range(CC)]

            def load_zh(j):
                ts_ = slice(j * P, (j + 1) * P)
                for cc in range(CC):
                    rows = slice(cc * P, (cc + 1) * P)
                    t = const_pool.tile([P, P], bf16, tag=f"zh{cc}j{j}",
                                        name=f"zh{cc}j{j}")
                    nc.sync.dma_start(out=t[:], in_=zh_ext[rows, ts_])
                    zh_sb[cc][j] = t

            def load_zl(j):
                ts_ = slice(j * P, (j + 1) * P)
                for cc in range(CC):
                    rows = slice(cc * P, (cc + 1) * P)
                    t = const_pool.tile([P, P], bf16, tag=f"zl{cc}j{j}",
                                        name=f"zl{cc}j{j}")
                    nc.sync.dma_start(out=t[:], in_=zl_ext[rows, ts_])
                    zl_sb[cc][j] = t

            def load_z(j):
                load_zh(j)
                load_zl(j)

            negB_row = const_pool.tile([1, K], f32, tag="negBrow")
            nc.sync.dma_start(out=negB_row[:], in_=negB_ext[:, :])
            # first accumulation group's biggest dependency first
            for cc in range(CC):
                rows = slice(cc * P, (cc + 1) * P)
                th = const_pool.tile([P, KT], bf16, tag=f"c2h{cc}k0",
                                     name=f"c2h{cc}k0e")
                nc.sync.dma_start(out=th[:], in_=c2h_ext[rows, 0:KT])
                c2h_sb[cc][0] = th
            load_z(0)
            negA_sb = const_pool.tile([P, NT], f32, tag="negA")
            nc.sync.dma_start(out=negA_sb[:], in_=negA_ext[:, :])
            negB_sb = [None] * NKT
            for kt in range(NKT):
                negB_sb[kt] = const_pool.tile([P, KT], f32,
                                              tag=f"negBk{kt}",
                                              name=f"negBk{kt}")
                nc.gpsimd.partition_broadcast(
                    negB_sb[kt][:],
                    negB_row[:, kt * KT:(kt + 1) * KT])
            for kt in range(NKT):
                ks = slice(kt * KT, (kt + 1) * KT)
                for cc in range(CC):
                    if kt == 0:
                        break
                    rows = slice(cc * P, (cc + 1) * P)
                    th = const_pool.tile([P, KT], bf16, tag=f"c2h{cc}k{kt}")
                    nc.sync.dma_start(out=th[:], in_=c2h_ext[rows, ks])
                    c2h_sb[cc][kt] = th
                for cc in range(CC):
                    rows = slice(cc * P, (cc + 1) * P)
                    tl = const_pool.tile([P, KT], bf16, tag=f"c2l{cc}k{kt}")
                    nc.sync.dma_start(out=tl[:], in_=c2l_ext[rows, ks])
                    c2l_sb[cc][kt] = tl
                if kt == 0:
                    load_z(1)

            for j in range(2, NT):
                load_z(j)

            mxa_d, ixa_d, best_d = {}, {}, {}

            def emit_step(j, nd, kt):
                ks = slice(kt * KT, (kt + 1) * KT)
                # nd slice = t1n = fl(negA + negB)  (one rounded add,
                # mirroring the reference's A+B broadcast add)
                nc.vector.tensor_scalar(
                    out=nd[:, ks], in0=negB_sb[kt][:],
                    scalar1=negA_sb[:, j:j + 1], scalar2=None, op0=add,
                )
                # pass order: all zh@c2h chunks first, so the group can
                # start once the c2h k-tile lands (c2l streams behind).
                # PSUM accumulation reorder shifts rounding only at the
                # ~1e-8 level, 100x under the argmin flip threshold.
                ps = mm_ps_pool.tile([P, KT], f32, tag="mm",
                                     name=f"mm{j}_{kt}")
                for cc in range(CC):
                    nc.tensor.matmul(
                        out=ps[:], lhsT=zh_sb[cc][j][:],
                        rhs=c2h_sb[cc][kt][:],
                        start=(cc == 0), stop=False,
                    )
                for cc in range(CC):
                    nc.tensor.matmul(
                        out=ps[:], lhsT=zh_sb[cc][j][:],
                        rhs=c2l_sb[cc][kt][:],
                        start=False, stop=False,
                    )
                for cc in range(CC):
                    nc.tensor.matmul(
                        out=ps[:], lhsT=zl_sb[cc][j][:],
                        rhs=c2h_sb[cc][kt][:],
                        start=False, stop=(cc == CC - 1),
                    )
                # nd = fl(t1n + 2m): the reference's second rounded add
                nc.vector.tensor_tensor(
                    out=nd[:, ks], in0=ps[:], in1=nd[:, ks], op=add,
                )
                # argmax (= argmin of d) of finished parts overlaps the
                # remaining matmuls. Normal tiles: one 2048-wide pass at
                # the halfway point. Last tile: 1024-wide quarters with
                # rolling merges, so only a quarter reduction and one
                # tiny merge trail the final matmul. All merges keep
                # first-occurrence tie-break: the earlier (lower-index)
                # part wins on equal values.
                if j < NT - 1:
                    if kt == NKT // 2 - 1:
                        HK = K // 2
                        mxa = small_pool.tile([P, 8], f32, tag="mxa")
                        ixa = small_pool.tile([P, 8], u32, tag="ixa")
                        nc.vector.max(out=mxa[:], in_=nd[:, 0:HK])
                        nc.vector.max_index(out=ixa[:], in_max=mxa[:],
                                            in_values=nd[:, 0:HK])
                        mxa_d[j], ixa_d[j] = mxa, ixa
                elif kt % 2 == 1:
                    q = kt // 2
                    qs = slice(q * 2 * KT, (q + 1) * 2 * KT)
                    mq = small_pool.tile([P, 8], f32, tag=f"mq{q}",
                                         name=f"mq{q}")
                    iq = small_pool.tile([P, 8], u32, tag=f"iq{q}",
                                         name=f"iq{q}")
                    nc.vector.max(out=mq[:], in_=nd[:, qs])
                    nc.vector.max_index(out=iq[:], in_max=mq[:],
                                        in_values=nd[:, qs])
                    if q == 0:
                        bestv = small_pool.tile([P, 1], f32, tag="bestv")
                        besti = small_pool.tile([P, 1], u32, tag="besti")
                        nc.vector.tensor_copy(out=bestv[:], in_=mq[:, 0:1])
                        nc.vector.tensor_copy(out=besti[:], in_=iq[:, 0:1])
                        best_d[j] = (bestv, besti)
                    else:
                        bestv, besti = best_d[j]
                        # merged = (bestv < mq) ? iq+off : besti;
                        # strict less-than keeps the earlier (lower
                        # index) part on ties
                        goff = small_pool.tile([P, 1], u32, tag=f"go{q}",
                                               name=f"go{q}")
                        nc.vector.tensor_scalar(
                            out=goff[:], in0=iq[:, 0:1],
                            scalar1=q * 2 * KT, scalar2=None, op0=add)
                        lmask = small_pool.tile([P, 1], u32, tag=f"lm{q}",
                                                name=f"lm{q}")
                        nc.vector.tensor_tensor(
                            out=lmask[:], in0=bestv[:], in1=mq[:, 0:1],
                            op=mybir.AluOpType.is_lt)
                        nc.vector.copy_predicated(
                            out=besti[:], mask=lmask[:], data=goff[:])
                        nc.vector.tensor_tensor(
                            out=bestv[:], in0=bestv[:], in1=mq[:, 0:1],
                            op=mybir.AluOpType.max)

            def emit_epilogue(j, nd):
                HK = K // 2
                if j < NT - 1:
                    # second-half reduction + merge
                    mxa, ixa = mxa_d[j], ixa_d[j]
                    mxb = small_pool.tile([P, 8], f32, tag="mxb")
                    ixb = small_pool.tile([P, 8], u32, tag="ixb")
                    nc.vector.max(out=mxb[:], in_=nd[:, HK:K])
                    nc.vector.max_index(out=ixb[:], in_max=mxb[:],
                                        in_values=nd[:, HK:K])
                    mask = small_pool.tile([P, 1], u32, tag="mask")
                    nc.vector.tensor_tensor(out=mask[:], in0=mxa[:, 0:1],
                                            in1=mxb[:, 0:1],
                                            op=mybir.AluOpType.is_ge)
                    idx = small_pool.tile([P, 1], u32, tag="idx")
                    nc.vector.tensor_scalar(
                        out=idx[:], in0=ixb[:, 0:1], scalar1=HK,
                        scalar2=None, op0=add)
                    nc.vector.copy_predicated(out=idx[:], mask=mask[:],
                                              data=ixa[:, 0:1])
                else:
                    idx = best_d[j][1]

                # The reference's decoder_input = ze + fl(zq - ze) differs
                # from zq only by f32 rounding at |ze| scale (~2.4e-7
                # absolute, 2.2e-5 global rel err) — emit zq directly.
                zq = ste_pool.tile([P, C], f32, tag="zq")
                nc.gpsimd.indirect_dma_start(
                    out=zq[:], out_offset=None,
                    in_=cb_ext[:],
                    in_offset=bass.IndirectOffsetOnAxis(ap=idx[:, :],
                                                        axis=0),
                )
                nc.sync.dma_start(out=out_ext[j * P:(j + 1) * P, :],
                                  in_=zq[:])

            # Tiles 0 and 1 interleave per k-tile: each arriving codebook
            # k-tile feeds two accumulation groups, halving the DMA
            # bandwidth pressure in the cold-start window.
            nd0 = nd_pool.tile([P, K], f32, tag="nd", name="nd0")
            nd1 = nd_pool.tile([P, K], f32, tag="nd", name="nd1")
            for kt in range(NKT):
                emit_step(0, nd0, kt)
                emit_step(1, nd1, kt)
            emit_epilogue(0, nd0)
            emit_epilogue(1, nd1)
            for j in range(2, NT):
                nd = nd_pool.tile([P, K], f32, tag="nd", name=f"nd{j}")
                for kt in range(NKT):
                    emit_step(j, nd, kt)
                emit_epilogue(j, nd)

    nc.compile()
    return nc


_NC_CACHE = None


def _get_graph():
    global _NC_CACHE
    if _NC_CACHE is None:
        _NC_CACHE = _build_graph()
    return _NC_CACHE


def _prep_inputs(feature: np.ndarray, codebook_w: np.ndarray):
    feature = np.asarray(feature, dtype=np.float32)
    codebook_w = np.asarray(codebook_w, dtype=np.float32)

    cb2t = np.ascontiguousarray((2.0 * codebook_w).T)          # [C, K] f32
    c2h = cb2t.astype(_BF16)
    c2l = (cb2t - c2h.astype(np.float32)).astype(_BF16)
    negB = -np.sum(codebook_w * codebook_w, axis=1, dtype=np.float32)  # [K]
    negb1 = np.ascontiguousarray(negB.reshape(1, K))

    in_maps = []
    for i in range(NCORES):
        n = i // 2
        h0 = (i % 2) * (H // 2)
        zeT = np.ascontiguousarray(
            feature[n, :, h0:h0 + H // 2, :].reshape(C, TC))
        zh = zeT.astype(_BF16)
        zl = (zeT - zh.astype(np.float32)).astype(_BF16)
        negA = -np.sum(zeT * zeT, axis=0, dtype=np.float32)    # [TC]
        negA_tiles = np.ascontiguousarray(negA.reshape(NT, P).T)  # [P, NT]
        in_maps.append({
            "zh": zh, "zl": zl,
            "c2h": c2h, "c2l": c2l,
            "negb1": negb1, "negA": negA_tiles,
            "cb": codebook_w,
        })
    return in_maps


def kernel(feature: np.ndarray, codebook_w: np.ndarray) -> np.ndarray:
    from concourse.bass_utils import run_bass_kernel_spmd

    nc = _get_graph()
    in_maps = _prep_inputs(feature, codebook_w)
    res = run_bass_kernel_spmd(nc, in_maps, core_ids=list(range(NCORES)))
    out = np.concatenate(
        [np.asarray(res.results[i]["out"]) for i in range(NCORES)], axis=0)
    return out



# revision 4
# speedup vs baseline: 1.4065x; 1.4065x over previous
"""VQ codebook-lookup kernel for one TRN2 chip (8 NeuronCores, SPMD).

Token-parallel sharding: the flattened token axis N*H*W = 16384 is split
into 8 shards of 2048 tokens; the [4096, 512] codebook is replicated.
Each core computes its own distances, argmin, gather; no collectives.

Two-stage argmin (approx rank + exact refine):

Stage 1 (rank): one fp16 matmul pass computes 2m ~= 2*ze@c per token
tile into PSUM. fp16 x fp16 products are exact in the PE's FP22/e10m23
pipeline, so the only stage-1 error is the host-side fp16 input
quantization (sigma ~ 4e-5 on distances) plus the fp16 rounding of the
PSUM->SBUF copy (~6e-5) and the dropped |c|^2 term (sigma 2.6e-5).
argmax_k of 2m ranks candidates; the true argmin's rank was measured
rank<=1 on all 16384 tokens, and P(rank >= 3) ~ 1e-6 analytically, so
a top-3 refine recovers the exact argmin.

Stage 2 (refine): MAX8/FIND_INDEX8 give the top-3 candidate indices.
For each candidate we gather [2*c_k | -B_k] from an augmented DRAM
table and replicate the reference's f32 rounding sequence:
    nd_i = fl( fl(-A_t + -B_k) + dot(ze_t, 2*c_k) )
(the negation of the reference's fl(fl(A+B) - 2m), exact by RN sign
symmetry). The f32 dot differs from the reference's f32 matmul by
~1.5e-8 (both accumulate-in-order errors), flipping ties only at gaps
< 3e-8: ~0.07 expected tokens. Winner by max with smaller-k tie-break.

The reference's straight-through output ze + fl(zq - ze) equals the
gathered codebook row zq up to one f32 rounding at |ze| scale (2.2e-5
global relative error, 1000x inside the accuracy gate), so the kernel
gathers and emits zq directly.
"""

import sys

for _p in ("/opt/trn_rl_repo", "/root/.axon_site/_ro/trn_rl_repo"):
    if _p not in sys.path:
        sys.path.insert(0, _p)

import numpy as np

N = 4
C = 512
H = 64
W = 64
K = 4096
T = N * H * W          # 16384 tokens
NCORES = 8
TC = T // NCORES       # 2048 tokens per core
P = 128                # partition tile
NT = TC // P           # 16 token tiles per core
KT = 512               # k-tile width (one PSUM bank)
NKT = K // KT          # 8 k tiles
CC = C // P            # 4 contraction chunks
TOPK = 3               # refined candidates per token
AUGW = 516             # aug row: 2*c (512) | -B (1) | pad (3)


def _build_graph():
    import concourse.bass as bass
    import concourse.mybir as mybir
    from concourse import bacc
    from concourse.tile import TileContext

    f32 = mybir.dt.float32
    fp16 = mybir.dt.float16
    u32 = mybir.dt.uint32
    add = mybir.AluOpType.add
    mult = mybir.AluOpType.mult

    nc = bacc.Bacc("TRN2", target_bir_lowering=False, debug=False,
                   num_devices=NCORES)

    z16_ext = nc.dram_tensor("z16", [C, TC], fp16, kind="ExternalInput").ap()
    c16_ext = nc.dram_tensor("c16", [C, K], fp16, kind="ExternalInput").ap()
    zet_ext = nc.dram_tensor("zet", [TC, C], f32, kind="ExternalInput").ap()
    negA_ext = nc.dram_tensor("negA", [P, NT], f32, kind="ExternalInput").ap()
    aug_ext = nc.dram_tensor("aug", [K, AUGW], f32, kind="ExternalInput").ap()
    cb_ext = nc.dram_tensor("cb", [K, C], f32, kind="ExternalInput").ap()
    out_ext = nc.dram_tensor("out", [TC, C], f32, kind="ExternalOutput").ap()

    with TileContext(nc) as tc:
        with (
            tc.tile_pool(name="const", bufs=1) as const_pool,
            tc.tile_pool(name="nd", bufs=3) as nd_pool,
            tc.tile_pool(name="small", bufs=4) as small_pool,
            tc.tile_pool(name="slots", bufs=2) as slots_pool,
            tc.tile_pool(name="ste", bufs=3) as ste_pool,
            tc.tile_pool(name="mm_ps", bufs=8, space="PSUM") as mm_ps_pool,
        ):
            z16_sb = [[None] * NT for _ in range(CC)]
            c16_sb = [[None] * NKT for _ in range(CC)]
            zet_sb = [None] * NT

            def load_z(j):
                ts_ = slice(j * P, (j + 1) * P)
                for cc in range(CC):
                    rows = slice(cc * P, (cc + 1) * P)
                    t = const_pool.tile([P, P], fp16, tag=f"z{cc}j{j}",
                                        name=f"z{cc}j{j}")
                    nc.sync.dma_start(out=t[:], in_=z16_ext[rows, ts_])
                    z16_sb[cc][j] = t

            def load_zet(j):
                t = const_pool.tile([P, C], f32, tag=f"zet{j}",
                                    name=f"zet{j}")
                nc.sync.dma_start(out=t[:],
                                  in_=zet_ext[j * P:(j + 1) * P, :])
                zet_sb[j] = t

            def load_c(kt):
                ks = slice(kt * KT, (kt + 1) * KT)
                for cc in range(CC):
                    rows = slice(cc * P, (cc + 1) * P)
                    t = const_pool.tile([P, KT], fp16, tag=f"c{cc}k{kt}",
                                        name=f"c{cc}k{kt}")
                    nc.sync.dma_start(out=t[:], in_=c16_ext[rows, ks])
                    c16_sb[cc][kt] = t

            # Cold start: first k-tile of the codebook, first two token
            # tiles, then the rest interleaved so early matmul groups
            # only depend on small DMAs.
            load_c(0)
            load_z(0)
            negA_sb = const_pool.tile([P, NT], f32, tag="negA")
            nc.sync.dma_start(out=negA_sb[:], in_=negA_ext[:, :])
            load_zet(0)
            load_z(1)
            for kt in range(1, NKT):
                load_c(kt)
                if kt == 1:
                    load_zet(1)
            for j in range(2, NT):
                load_z(j)
                load_zet(j)

            def emit_step(j, nd16, kt):
                # 2m accumulation: four fp16 chunk matmuls into one bank
                ps = mm_ps_pool.tile([P, KT], f32, tag="mm",
                                     name=f"mm{j}_{kt}")
                for cc in range(CC):
                    nc.tensor.matmul(
                        out=ps[:], lhsT=z16_sb[cc][j][:],
                        rhs=c16_sb[cc][kt][:],
                        start=(cc == 0), stop=(cc == CC - 1),
                    )
                # PSUM -> SBUF as fp16 on the (otherwise idle) scalar
                # engine; fp16 halves the DVE max/find cost below.
                nc.scalar.copy(out=nd16[:, kt * KT:(kt + 1) * KT],
                               in_=ps[:])

            def emit_refine(j, nd16):
                mx8 = small_pool.tile([P, 8], fp16, tag="mx8",
                                      name=f"mx8_{j}")
                ix8 = small_pool.tile([P, 8], u32, tag="ix8",
                                      name=f"ix8_{j}")
                nc.vector.max(out=mx8[:], in_=nd16[:])
                nc.vector.max_index(out=ix8[:], in_max=mx8[:],
                                    in_values=nd16[:])

                bestv = small_pool.tile([P, 1], f32, tag="bestv",
                                        name=f"bestv{j}")
                besti = small_pool.tile([P, 1], u32, tag="besti",
                                        name=f"besti{j}")
                for s in range(TOPK):
                    slot = slots_pool.tile([P, AUGW], f32, tag=f"slot{s}",
                                           name=f"slot{s}_{j}")
                    nc.gpsimd.indirect_dma_start(
                        out=slot[:], out_offset=None,
                        in_=aug_ext[:],
                        in_offset=bass.IndirectOffsetOnAxis(
                            ap=ix8[:, s:s + 1], axis=0),
                    )
                    # s = dot(ze_t, 2c_k) in f32: DVE multiply, then
                    # add-reduce on the scalar engine (activation Copy
                    # with accum_out). tensor_tensor_reduce would fuse
                    # both, but it crashes TRN2 hardware.
                    scratch = slots_pool.tile([P, C], f32, tag=f"scr{s}",
                                              name=f"scr{s}_{j}")
                    nc.vector.tensor_tensor(
                        out=scratch[:], in0=slot[:, 0:C],
                        in1=zet_sb[j][:], op=mult)
                    scr2 = slots_pool.tile([P, C], f32, tag=f"scr2_{s}",
                                           name=f"scr2_{s}_{j}")
                    ssum = small_pool.tile([P, 1], f32, tag=f"ss{s}",
                                           name=f"ss{s}_{j}")
                    nc.scalar.activation(
                        out=scr2[:], in_=scratch[:],
                        func=mybir.ActivationFunctionType.Copy,
                        accum_out=ssum[:])
                    # the reference's two rounded adds (negated), fused:
                    # nd = fl( fl(-B + -A) + 2m )
                    nds = small_pool.tile([P, 1], f32, tag=f"nds{s}",
                                          name=f"nds{s}_{j}")
                    nc.vector.scalar_tensor_tensor(
                        out=nds[:], in0=slot[:, C:C + 1],
                        scalar=negA_sb[:, j:j + 1], in1=ssum[:],
                        op0=add, op1=add)
                    if s == 0:
                        nc.vector.tensor_copy(out=bestv[:], in_=nds[:])
                        nc.vector.tensor_copy(out=besti[:],
                                              in_=ix8[:, 0:1])
                    else:
                        # take = (nds > bestv) or (nds == bestv and
                        # ix_s < besti)  [smaller-k tie-break]
                        gt = small_pool.tile([P, 1], u32, tag=f"gt{s}",
                                             name=f"gt{s}_{j}")
                        nc.vector.tensor_tensor(
                            out=gt[:], in0=nds[:], in1=bestv[:],
                            op=mybir.AluOpType.is_gt)
                        eq = small_pool.tile([P, 1], u32, tag=f"eq{s}",
                                             name=f"eq{s}_{j}")
                        nc.vector.tensor_tensor(
                            out=eq[:], in0=nds[:], in1=bestv[:],
                            op=mybir.AluOpType.is_equal)
                        ltk = small_pool.tile([P, 1], u32, tag=f"lt{s}",
                                              name=f"lt{s}_{j}")
                        nc.vector.tensor_tensor(
                            out=ltk[:], in0=ix8[:, s:s + 1],
                            in1=besti[:], op=mybir.AluOpType.is_lt)
                        nc.vector.tensor_tensor(
                            out=eq[:], in0=eq[:], in1=ltk[:],
                            op=mybir.AluOpType.logical_and)
                        nc.vector.tensor_tensor(
                            out=gt[:], in0=gt[:], in1=eq[:],
                            op=mybir.AluOpType.logical_or)
                        nc.vector.copy_predicated(out=bestv[:],
                                                  mask=gt[:],
                                                  data=nds[:])
                        nc.vector.copy_predicated(out=besti[:],
                                                  mask=gt[:],
                                                  data=ix8[:, s:s + 1])

                zq = ste_pool.tile([P, C], f32, tag="zq", name=f"zq{j}")
                nc.gpsimd.indirect_dma_start(
                    out=zq[:], out_offset=None,
                    in_=cb_ext[:],
                    in_offset=bass.IndirectOffsetOnAxis(ap=besti[:, :],
                                                        axis=0),
                )
                nc.sync.dma_start(out=out_ext[j * P:(j + 1) * P, :],
                                  in_=zq[:])

            # Tiles 0 and 1 interleave per k-tile so each arriving
            # codebook k-tile feeds two accumulation groups during the
            # cold-start window.
            nd0 = nd_pool.tile([P, K], fp16, tag="nd", name="nd0")
            nd1 = nd_pool.tile([P, K], fp16, tag="nd", name="nd1")
            for kt in range(NKT):
                emit_step(0, nd0, kt)
                emit_step(1, nd1, kt)
            emit_refine(0, nd0)
            emit_refine(1, nd1)
            for j in range(2, NT):
                nd = nd_pool.tile([P, K], fp16, tag="nd", name=f"nd{j}")
                for kt in range(NKT):
                    emit_step(j, nd, kt)
                emit_refine(j, nd)

    nc.compile()
    return nc


_NC_CACHE = None


def _get_graph():
    global _NC_CACHE
    if _NC_CACHE is None:
        _NC_CACHE = _build_graph()
    return _NC_CACHE


def _prep_inputs(feature: np.ndarray, codebook_w: np.ndarray):
    feature = np.asarray(feature, dtype=np.float32)
    codebook_w = np.asarray(codebook_w, dtype=np.float32)

    c2t = np.ascontiguousarray((2.0 * codebook_w).T)           # [C, K] f32
    c16 = c2t.astype(np.float16)
    negB = -np.sum(codebook_w * codebook_w, axis=1, dtype=np.float32)
    aug = np.zeros((K, AUGW), dtype=np.float32)
    aug[:, 0:C] = 2.0 * codebook_w
    aug[:, C] = negB

    in_maps = []
    for i in range(NCORES):
        n = i // 2
        h0 = (i % 2) * (H // 2)
        zeT = np.ascontiguousarray(
            feature[n, :, h0:h0 + H // 2, :].reshape(C, TC))
        z16 = zeT.astype(np.float16)
        zet = np.ascontiguousarray(zeT.T)                      # [TC, C]
        negA = -np.sum(zeT * zeT, axis=0, dtype=np.float32)    # [TC]
        negA_tiles = np.ascontiguousarray(negA.reshape(NT, P).T)
        in_maps.append({
            "z16": z16, "c16": c16, "zet": zet,
            "negA": negA_tiles, "aug": aug, "cb": codebook_w,
        })
    return in_maps


def kernel(feature: np.ndarray, codebook_w: np.ndarray) -> np.ndarray:
    from concourse.bass_utils import run_bass_kernel_spmd

    nc = _get_graph()
    in_maps = _prep_inputs(feature, codebook_w)
    res = run_bass_kernel_spmd(nc, in_maps, core_ids=list(range(NCORES)))
    out = np.concatenate(
        [np.asarray(res.results[i]["out"]) for i in range(NCORES)], axis=0)
    return out
